# revision 14
# baseline (speedup 1.0000x reference)
"""3-layer GraphSAGE (mean aggregation) on 8 Trainium2 NeuronCores.

Sharding: destination nodes are partitioned across the 8 cores (Cluster-GCN
style node sharding); features and weights are replicated.  Per layer, each
core gathers the (bf16) source-node rows for its shard's edges with
dma_gather, segment-sums them on the tensor engine via one-hot matmuls into
PSUM, applies 1/deg, and runs the dense lin_l/lin_r matmuls with the weights
stationary (out^T layout).  An AllGather replicates the new hidden state for
the next layer's gather.  All graph preprocessing (edge sorting/padding,
int16 gather indices, rebased dst ids, degrees) happens on the host in
numpy; the device program is identical across cores (SPMD) with per-core
data supplied through input tensors.

Performance structure:
- Gather descriptor generation (SWDGE on the GpSimd Q7 cores) is the
  critical path; gather calls round-robin the 4 SWDGE queues so descriptor
  generation for consecutive calls overlaps on different Q7 core pairs.
- The replicated node-feature table is split into two contiguous regions:
  E = rows [0:SPLIT) of every shard, L = rows [SPLIT:SHARD_P) of every
  shard, each its own DRAM tensor replicated via its own AllGather.  Each
  layer aggregates E-region edges first (copy into aggT), then L-region
  edges (add into aggT).  AG_E(l) fires mid-layer (as soon as the first
  SPLIT h rows exist) and AG_L(l) at layer end, so AG_L(l) overlaps the
  next layer's E-edge gathers.
- Dense matmuls + h-row transposes are interleaved into the aggregation
  tile loop (groups of 4 tiles) to run in the tensor engine's idle slack
  under the gather stream.
"""

import os
import sys

sys.path.insert(0, "/opt/trn_rl_repo")

import numpy as np
import ml_dtypes

from concourse import bass, bacc, mybir, library_config
import concourse.tile as tile
from concourse.bass_utils import run_bass_kernel_spmd

BF16 = mybir.dt.bfloat16
F32 = mybir.dt.float32
I16 = mybir.dt.int16
FP8 = mybir.dt.float8e4
NP_BF16 = ml_dtypes.bfloat16
NP_FP8 = ml_dtypes.float8_e4m3fn

P = 128


class Cfg:
    def __init__(self, n=50000, e=800000, d=256, out_d=64, cores=8):
        self.N = n
        self.E = e
        self.D = d            # in/hidden dim (256)
        self.OUT_D = out_d    # final dim (64)
        self.C = cores
        assert n % cores == 0
        self.SHARD = n // cores
        self.TILES = (self.SHARD + P - 1) // P
        self.SHARD_P = self.TILES * P
        self.NP = self.C * self.SHARD_P
        # E/L split of each shard's rows (E gets SPLIT_T tiles)
        self.SPLIT_T = 19
        self.SPLIT = self.SPLIT_T * P
        self.RE = self.SPLIT                   # E rows per core
        self.RL = self.SHARD_P - self.SPLIT    # L rows per core (3072)
        self.NE = self.C * self.RE
        self.NL = self.C * self.RL
        assert self.NE <= 32768 and self.NL <= 32768, "gather idx fits int16"
        self.KC = self.D // P  # k chunks of the 256-dim (2)


class Structure:
    """Program structure shared by all cores (derived from max counts)."""

    def __init__(self, cfg, nb, b_call=24):
        # nb[t][r] = number of 128-edge blocks for dst tile t, region r (E/L)
        self.nb = nb
        self.b_call = b_call
        self.block_col = {}  # (t, r) -> start block col within region stream
        self.tb = [0, 0]
        for h in (0, 1):
            col = 0
            for t in range(cfg.TILES):
                self.block_col[(t, h)] = col
                col += nb[t][h]
            self.tb[h] = col
        self.calls = [(tb + b_call - 1) // b_call for tb in self.tb]
        self.total_blocks = self.tb[0] + self.tb[1]  # real blocks (dstreb cols)
        # int16 idx array layout: region-E stream then region-L stream, each
        # padded to calls*b_call blocks; 8 int16 cols per block (128/16)
        self.idx_off = [0, self.calls[0] * b_call * 8]
        self.idx_w = (self.calls[0] + self.calls[1]) * b_call * 8

    def reb_col(self, t, h, b):
        return (self.tb[0] if h else 0) + self.block_col[(t, h)] + b


def preprocess(x, edge_index, cfg, b_call=24):
    """Host-side numpy preprocessing. Returns (structure, shared, per_core)."""
    src = np.asarray(edge_index[0], dtype=np.int64)
    dst = np.asarray(edge_index[1], dtype=np.int64)

    shard_of = dst // cfg.SHARD
    dst_local = dst % cfg.SHARD
    tile_of = dst_local // P
    reb = dst_local % P
    # source row split into E/L regions of the replicated table
    sc = src // cfg.SHARD
    sr = src % cfg.SHARD
    reg = (sr >= cfg.SPLIT).astype(np.int64)
    idx16 = np.where(reg == 0, sc * cfg.RE + sr, sc * cfg.RL + (sr - cfg.SPLIT))

    # counts per (core, tile, region)
    key = ((shard_of * cfg.TILES + tile_of) * 2 + reg).astype(np.int64)
    nkeys = cfg.C * cfg.TILES * 2
    counts = np.bincount(key, minlength=nkeys).reshape(cfg.C, cfg.TILES, 2)
    kmax = counts.max(axis=0)  # [TILES, 2]
    kb = ((kmax + P - 1) // P).astype(np.int64)  # blocks, may be 0
    nb = [[int(kb[t, 0]), int(kb[t, 1])] for t in range(cfg.TILES)]
    S = Structure(cfg, nb, b_call=b_call)

    # sort edges by (core, tile, region, src) for locality
    order = np.lexsort((idx16, key))
    key_s = key[order]
    idx16_s = idx16[order]
    reb_s = reb[order]
    starts = np.searchsorted(key_s, np.arange(nkeys))
    ends = np.searchsorted(key_s, np.arange(nkeys) + 1)

    deg = np.bincount(dst, minlength=cfg.N).astype(np.float32)
    deginv_full = 1.0 / np.maximum(deg, 1.0)

    L = b_call * P  # idxs per call
    per_core = []
    for c in range(cfg.C):
        idx_all = np.zeros((P, S.idx_w), dtype=np.int16)
        reb_stream = np.full(S.total_blocks * P, P, dtype=np.float32)  # pad=128
        for h in (0, 1):
            stream = np.zeros(S.calls[h] * L, dtype=np.int16)
            for t in range(cfg.TILES):
                nblk = nb[t][h]
                if nblk == 0:
                    continue
                k = (c * cfg.TILES + t) * 2 + h
                s0, e0 = starts[k], ends[k]
                cnt = e0 - s0
                base = S.block_col[(t, h)] * P
                stream[base:base + cnt] = idx16_s[s0:e0].astype(np.int16)
                rbase = S.reb_col(t, h, 0) * P
                reb_stream[rbase:rbase + cnt] = reb_s[s0:e0].astype(np.float32)
            # wrap each call window: idx j -> [j%16, j//16], tiled over 128 rows
            for kcall in range(S.calls[h]):
                seg = stream[kcall * L:(kcall + 1) * L].reshape(L // 16, 16).T
                off = S.idx_off[h] + kcall * b_call * 8
                idx_all[:, off:off + L // 16] = np.tile(seg, (8, 1))
        dstreb = np.ascontiguousarray(
            reb_stream.reshape(S.total_blocks, P).T).astype(NP_BF16)  # [128, TB]

        dgi = np.ones((P, cfg.TILES), dtype=np.float32)
        dl = deginv_full[c * cfg.SHARD:(c + 1) * cfg.SHARD]
        dl_pad = np.concatenate([dl, np.ones(cfg.SHARD_P - cfg.SHARD, np.float32)])
        dgi[:, :] = dl_pad.reshape(cfg.TILES, P).T

        xs = np.asarray(x[c * cfg.SHARD:(c + 1) * cfg.SHARD], dtype=np.float32)
        xs_pad = np.zeros((cfg.SHARD_P, cfg.D), dtype=np.float32)
        xs_pad[:cfg.SHARD] = xs
        xT = np.ascontiguousarray(xs_pad.T).reshape(cfg.KC, P, cfg.SHARD_P)

        per_core.append(dict(
            idx_all=idx_all,
            dstreb=dstreb,
            deginv=dgi,
            xT_own=xT.astype(NP_BF16),
        ))

    # replicated x in E/L region layout
    xE = np.zeros((cfg.NE, cfg.D), dtype=NP_FP8)
    xL = np.zeros((cfg.NL, cfg.D), dtype=NP_FP8)
    for c in range(cfg.C):
        rows = np.asarray(x[c * cfg.SHARD:(c + 1) * cfg.SHARD]).astype(NP_FP8)
        xE[c * cfg.RE:(c + 1) * cfg.RE] = rows[:cfg.SPLIT]
        xL[c * cfg.RL:c * cfg.RL + (cfg.SHARD - cfg.SPLIT)] = rows[cfg.SPLIT:]

    iota = np.broadcast_to(np.arange(P, dtype=np.float32), (P, P))
    shared = dict(
        xE=xE,
        xL=xL,
        iota=np.ascontiguousarray(iota).astype(NP_BF16),
        ident=np.eye(P, dtype=np.float32).astype(NP_BF16),
    )
    return S, shared, per_core


def pack_weights(cfg, Ws):
    """Ws: dict with Wl0..b2 from setup_inputs. Returns name->array (shared)."""
    out = {}
    douts = [cfg.D, cfg.D, cfg.OUT_D]
    bias = np.zeros((P, 5), dtype=np.float32)
    bcol = 0
    for l in range(3):
        do = douts[l]
        for nm in ("Wl", "Wr"):
            w = np.asarray(Ws[f"{nm}{l}"], dtype=np.float32)  # [D, do]
            out[f"{nm}{l}"] = np.ascontiguousarray(
                w.reshape(cfg.KC, P, do)).astype(NP_BF16)
        b = np.asarray(Ws[f"b{l}"], dtype=np.float32)
        nco = (do + P - 1) // P
        for co in range(nco):
            seg = b[co * P:(co + 1) * P]
            bias[:len(seg), bcol] = seg
            bcol += 1
    out["bias"] = bias
    return out


def build(cfg, S, n_layers=3):
    """Build the SPMD bass program (identical for all cores)."""
    nc = bacc.Bacc("TRN2", target_bir_lowering=False, debug=False,
                   num_devices=cfg.C, num_swdge_queues=4)
    douts = [cfg.D, cfg.D, cfg.OUT_D]
    BC = S.b_call
    L = BC * P

    # ---- DRAM parameters
    xE = nc.declare_dram_parameter("xE", [cfg.NE, cfg.D], FP8, isOutput=False)
    xL = nc.declare_dram_parameter("xL", [cfg.NL, cfg.D], FP8, isOutput=False)
    xT_own = nc.declare_dram_parameter("xT_own", [cfg.KC, P, cfg.SHARD_P], BF16, isOutput=False)
    idx_all = nc.declare_dram_parameter("idx_all", [P, S.idx_w], I16, isOutput=False)
    dstreb = nc.declare_dram_parameter("dstreb", [P, S.total_blocks], BF16, isOutput=False)
    deginv = nc.declare_dram_parameter("deginv", [P, cfg.TILES], F32, isOutput=False)
    iota = nc.declare_dram_parameter("iota", [P, P], BF16, isOutput=False)
    ident = nc.declare_dram_parameter("ident", [P, P], BF16, isOutput=False)
    wts = {}
    for l in range(3):
        for nm in ("Wl", "Wr"):
            wts[f"{nm}{l}"] = nc.declare_dram_parameter(
                f"{nm}{l}", [cfg.KC, P, douts[l]], BF16, isOutput=False)
    bias = nc.declare_dram_parameter("bias", [P, 5], F32, isOutput=False)
    outT = nc.declare_dram_parameter("outT", [cfg.OUT_D, cfg.SHARD_P], F32, isOutput=True)

    # ---- internal DRAM
    # h_shE/h_shL are separate tensors so AG_E's dependency tracking only
    # covers the E-tile phase-C writes (a single h_sh tensor made AG_E wait
    # for the whole layer's writes)
    h_shE = [nc.dram_tensor(f"h_shE{l}", [cfg.SPLIT, cfg.D], FP8) for l in (0, 1)]
    h_shL = [nc.dram_tensor(f"h_shL{l}", [cfg.RL, cfg.D], FP8) for l in (0, 1)]
    # NOTE: dma_gather from a Shared-scratchpad tensor hangs the device
    # (SWDGE address resolution), so the gather tables are Local tensors and
    # the AllGathers take the bounce path into them.  E and L regions are
    # separate tensors so the E gathers of layer l+1 only depend on AG_E(l).
    hE = [nc.dram_tensor(f"hE{l}", [cfg.NE, cfg.D], FP8) for l in (0, 1)]
    hL = [nc.dram_tensor(f"hL{l}", [cfg.NL, cfg.D], FP8) for l in (0, 1)]

    groups_all = [[c for c in range(cfg.C)]]

    with tile.TileContext(nc, num_cores=cfg.C) as tc:
        with (
            tc.tile_pool(name="const", bufs=1) as constp,
            tc.tile_pool(name="state", bufs=1) as statep,
            tc.tile_pool(name="msg", bufs=8) as msgp,
            tc.tile_pool(name="work", bufs=3) as workp,
            tc.tile_pool(name="psA", bufs=2, space="PSUM") as psA,
            tc.tile_pool(name="psT", bufs=2, space="PSUM") as psT,
            tc.tile_pool(name="psD", bufs=2, space="PSUM") as psD,
        ):
            reg_nidx = nc.gpsimd.to_reg(L)  # shared num_idxs register
            gq = [0]  # round-robin SWDGE queue counter (4 Q7 core pairs)

            # ---- load constants into SBUF
            def load(pool, ap, shape, dt, tag):
                t = pool.tile(shape, dt, tag=tag, name=tag)
                nc.sync.dma_start(out=t[:], in_=ap)
                return t

            oy = S.idx_off[1]
            idxE_sb = load(constp, idx_all[:, 0:oy], [P, oy], I16, "idxE")
            idxL_sb = load(constp, idx_all[:, oy:S.idx_w], [P, S.idx_w - oy],
                           I16, "idxL")
            reb_sb = load(constp, dstreb[:, :], [P, S.total_blocks], BF16, "reb")
            dgi_sb = load(constp, deginv[:, :], [P, cfg.TILES], F32, "dgi")
            iota_sb = load(constp, iota[:, :], [P, P], BF16, "iota")
            id_sb = load(constp, ident[:, :], [P, P], BF16, "ident")
            bias_sb = load(constp, bias[:, :], [P, 5], F32, "bias")
            w_sb = {}
            for l in range(3):
                for nm in ("Wl", "Wr"):
                    for ci in range(cfg.KC):
                        w_sb[(nm, l, ci)] = load(
                            constp, wts[f"{nm}{l}"][ci], [P, douts[l]], BF16,
                            f"{nm}{l}_{ci}")

            # persistent activation buffers (transposed layout, bf16)
            hT = [[statep.tile([P, cfg.SHARD_P], BF16, tag=f"hT{buf}_{ci}",
                               name=f"hT{buf}_{ci}")
                   for ci in range(cfg.KC)] for buf in (0, 1)]
            aggT = [statep.tile([P, cfg.SHARD_P], BF16, tag=f"aggT_{ci}",
                                name=f"aggT_{ci}")
                    for ci in range(cfg.KC)]
            for ci in range(cfg.KC):
                nc.sync.dma_start(out=hT[0][ci][:], in_=xT_own[ci])

            for l in range(n_layers):
                do = douts[l]
                nco = (do + P - 1) // P
                bias_col = [0, 2, 4][l]
                if l == 0:
                    halves = [xE[:, :], xL[:, :]]
                else:
                    halves = [hE[l - 1][:, :], hL[l - 1][:, :]]
                hT_cur = hT[l % 2]
                hT_nxt = hT[(l + 1) % 2]

                msg_tiles = {}

                def gather_call(h, kcall, halves=halves, msg_tiles=msg_tiles):
                    if (h, kcall) in msg_tiles:
                        return msg_tiles[(h, kcall)]
                    mt = msgp.tile([P, BC, cfg.D], FP8, tag="msg", name="msg")
                    off = kcall * BC * 8
                    isb = idxL_sb if h else idxE_sb
                    nc.gpsimd.dma_gather(
                        out_ap=mt[:],
                        in_ap=halves[h],
                        idxs_ap=isb[:, off:off + BC * 8],
                        num_idxs=L,
                        num_idxs_reg=reg_nidx,
                        elem_size=cfg.D,
                        # >64 descriptors per engine won't fit one packet
                        single_packet=False,
                        # round-robin the 4 SWDGE queues: desc-gen for queue q
                        # runs on Q7 core pair (2q, 2q+1), so consecutive
                        # calls' descriptor generation overlaps
                        queue_num=gq[0],
                    )
                    gq[0] = (gq[0] + 1) % 4
                    msg_tiles[(h, kcall)] = mt
                    return mt

                def accum_region(t, h):
                    """Scatter-matmul region h's blocks of dst tile t into a
                    PSUM tile; returns the deg-scaled bf16 [P, D] rows or
                    None if the region has no blocks for this tile."""
                    nbh = S.nb[t][h]
                    if nbh == 0:
                        return None
                    ps_full = psA.tile([P, 512], F32, tag="agg", name="ps")
                    ps = ps_full[:, :cfg.D]
                    oh = workp.tile([P, nbh, P], BF16, tag="oh", name="oh")
                    r0 = S.reb_col(t, h, 0)
                    nc.vector.tensor_tensor(
                        out=oh[:, :, :],
                        in0=iota_sb[:, None, :].to_broadcast([P, nbh, P]),
                        in1=reb_sb[:, r0:r0 + nbh, None].to_broadcast(
                            [P, nbh, P]),
                        op=mybir.AluOpType.is_equal,
                    )
                    c0 = S.block_col[(t, h)]
                    for b in range(nbh):
                        col = c0 + b
                        mt = gather_call(h, col // BC)
                        nc.tensor.matmul(
                            out=ps[:],
                            lhsT=oh[:, b, :],
                            rhs=mt[:, col % BC, :],
                            start=(b == 0),
                            stop=(b == nbh - 1),
                        )
                    agg_s = workp.tile([P, cfg.D], BF16, tag="agg_s",
                                       name="agg_s")
                    nc.vector.tensor_scalar_mul(
                        agg_s[:], ps[:], dgi_sb[:, t:t + 1])
                    return agg_s

                # ---- sub-phase E: aggregate E-region edges, copy into aggT
                for t in range(cfg.TILES):
                    agg_s = accum_region(t, 0)
                    if agg_s is None:
                        agg_s = workp.tile([P, cfg.D], BF16, tag="agg_s",
                                           name="agg_s")
                        nc.vector.memset(agg_s[:], 0.0)
                    for ci in range(cfg.KC):
                        pt = psT.tile([P, 1024], BF16, tag="tr", name="pt")
                        nc.tensor.transpose(
                            pt[:, :P], agg_s[:, ci * P:(ci + 1) * P], id_sb[:])
                        nc.vector.tensor_copy(
                            out=aggT[ci][:, t * P:(t + 1) * P], in_=pt[:, :P])

                # ---- sub-phase L: aggregate L-region edges, add into aggT;
                # dense matmuls + h rows interleave per group of 4 tiles
                for t in range(cfg.TILES):
                    agg_s = accum_region(t, 1)
                    if agg_s is not None:
                        for ci in range(cfg.KC):
                            pt = psT.tile([P, 1024], BF16, tag="tr", name="pt")
                            nc.tensor.transpose(
                                pt[:, :P], agg_s[:, ci * P:(ci + 1) * P],
                                id_sb[:])
                            nc.vector.tensor_tensor(
                                out=aggT[ci][:, t * P:(t + 1) * P],
                                in0=aggT[ci][:, t * P:(t + 1) * P],
                                in1=pt[:, :P],
                                op=mybir.AluOpType.add,
                            )
                    if t % 4 == 3 or t == cfg.TILES - 1:
                        g0 = (t // 4) * 4
                        s0 = g0 * P
                        w = (t - g0 + 1) * P
                        for co in range(nco):
                            m = min(P, do - co * P)
                            pd = psD.tile([P, 512], F32, tag="dense", name="pd")
                            for ci in range(cfg.KC):
                                nc.tensor.matmul(
                                    out=pd[:m, :w],
                                    lhsT=w_sb[("Wl", l, ci)][:, co * P:co * P + m],
                                    rhs=aggT[ci][:, s0:s0 + w],
                                    start=(ci == 0), stop=False,
                                )
                                nc.tensor.matmul(
                                    out=pd[:m, :w],
                                    lhsT=w_sb[("Wr", l, ci)][:, co * P:co * P + m],
                                    rhs=hT_cur[ci][:, s0:s0 + w],
                                    start=False, stop=(ci == cfg.KC - 1),
                                )
                            if l < 2:
                                nc.scalar.activation(
                                    out=hT_nxt[co][:m, s0:s0 + w], in_=pd[:m, :w],
                                    func=mybir.ActivationFunctionType.Relu,
                                    bias=bias_sb[:m, bias_col + co:bias_col + co + 1],
                                )
                            else:
                                ot = workp.tile([P, 512], F32, tag="outc", name="ot")
                                nc.scalar.activation(
                                    out=ot[:m, :w], in_=pd[:m, :w],
                                    func=mybir.ActivationFunctionType.Identity,
                                    bias=bias_sb[:m, bias_col + co:bias_col + co + 1],
                                )
                                nc.sync.dma_start(
                                    out=outT[co * P:co * P + m, s0:s0 + w],
                                    in_=ot[:m, :w])
                        if l < 2:
                            for tg in range(g0, t + 1):
                                hr = workp.tile([P, cfg.D], FP8, tag="hrow",
                                                name="hr")
                                for ci in range(cfg.KC):
                                    pt = psT.tile([P, 1024], BF16, tag="tr",
                                                  name="pt")
                                    nc.tensor.transpose(
                                        pt[:, :P],
                                        hT_nxt[ci][:, tg * P:(tg + 1) * P],
                                        id_sb[:])
                                    nc.vector.tensor_copy(
                                        out=hr[:, ci * P:(ci + 1) * P],
                                        in_=pt[:, :P])
                                if tg < cfg.SPLIT_T:
                                    nc.sync.dma_start(
                                        out=h_shE[l][tg * P:(tg + 1) * P, :],
                                        in_=hr[:])
                                else:
                                    tr = tg - cfg.SPLIT_T
                                    nc.sync.dma_start(
                                        out=h_shL[l][tr * P:(tr + 1) * P, :],
                                        in_=hr[:])
                            # AG_E fires as soon as the first SPLIT h rows
                            # exist; it overlaps the rest of this layer
                            if t + 1 >= cfg.SPLIT_T and g0 < cfg.SPLIT_T:
                                nc.gpsimd.collective_compute(
                                    "AllGather",
                                    mybir.AluOpType.bypass,
                                    replica_groups=groups_all,
                                    ins=[h_shE[l][:, :]],
                                    outs=[hE[l][:, :]],
                                )
                # AG_L at layer end; overlaps the next layer's E gathers
                if l < 2:
                    nc.gpsimd.collective_compute(
                        "AllGather",
                        mybir.AluOpType.bypass,
                        replica_groups=groups_all,
                        ins=[h_shL[l][:, :]],
                        outs=[hL[l][:, :]],
                    )
    nc.compile()
    return nc


def _ensure_ntff_hook():
    """Provide antenv.axon_hooks + register the ctypes NTFF hook if absent."""
    import types
    try:
        from antenv.axon_hooks import (
            get_axon_ntff_profile_hook, set_axon_ntff_profile_hook)
    except ImportError:
        import antenv
        mod = types.ModuleType("antenv.axon_hooks")
        mod._hook = None

        def _set(h):
            mod._hook = h

        def _get():
            return mod._hook

        mod.set_axon_ntff_profile_hook = _set
        mod.get_axon_ntff_profile_hook = _get
        sys.modules["antenv.axon_hooks"] = mod
        antenv.axon_hooks = mod
        get_axon_ntff_profile_hook, set_axon_ntff_profile_hook = _get, _set
    if get_axon_ntff_profile_hook() is None:
        try:
            from trn_agent_boot.trn_boot import _ntff_profile_via_ctypes
            h = _ntff_profile_via_ctypes("/opt/axon/libaxon_pjrt.so")
            if h is not None:
                set_axon_ntff_profile_hook(h)
        except Exception as e:
            print(f"ntff hook setup failed: {e}", file=sys.stderr)


def run(x, edge_index, weights, cfg=None, trace=False, b_call=24, n_layers=3):
    if trace:
        _ensure_ntff_hook()
    cfg = cfg or Cfg()
    S, shared, per_core = preprocess(x, edge_index, cfg, b_call=b_call)
    wpack = pack_weights(cfg, weights)
    nc = build(cfg, S, n_layers=n_layers)
    in_maps = []
    for c in range(cfg.C):
        m = dict(shared)
        m.update(per_core[c])
        m.update(wpack)
        in_maps.append(m)
    res = run_bass_kernel_spmd(nc, in_maps, list(range(cfg.C)), trace=trace)
    outs = []
    for c in range(cfg.C):
        oT = res.results[c]["outT"]  # [OUT_D, SHARD_P]
        outs.append(np.ascontiguousarray(oT.T[:cfg.SHARD, :]))
    full = np.concatenate(outs, axis=0).astype(np.float32)
    return full, res


def kernel(**inputs):
    x = inputs["x"]
    edge_index = inputs["edge_index"]
    weights = {k: inputs[k] for k in inputs if k not in ("x", "edge_index")}
    out, _ = run(x, edge_index, weights)
    return out
